# revision 26
# baseline (speedup 1.0000x reference)
"""Trainium2 Bass kernel: dual-softmax ("contrast") multi-head self-attention.

Problem (per full input):
  x, y: (4, 1024, 1024) f32; Wq/Wk/Wv: (1024, 1024) f32, nh=16 heads, dk=dv=64.
  q = x @ Wq.T, k = x @ Wk.T, v = y @ Wv.T  (split heads)
  dist   = softmax(q k^T / 8)
  c_att  = softmax(1 - dist) @ v      (== softmax(-dist) @ v, shift invariance)
  att    = softmax(dist) @ v
  returns (c_att, att), each (4, 1024, 1024) f32.

Sharding: 8 cores = 4 batches x 2 head-groups (8 heads each). Each core gets
x[b], y[b] and a 512-row slice of each weight; returns (c_att, att) slices
[1024, 512].

V2 design (bf16 matmuls, transpose-free scores):
  Loads arrive pre-transposed: DMA f32 -> DVE cast bf16 -> SBUF->SBUF
  dma_start_transpose (xbar) -> xt/yt/wqt/wkt/wvt in [contract-dim, free] form.
  QT = wqt.T @ xt, KT = wkt.T @ xt   [feat, tok] bf16; V via lhsT=yt slices.
  Per head, k-major throughout (no PE transposes of the score matrix):
    S^T[k,q] = KT_h^T QT_h via matmul (16 x 512-col bf16 MMs)
    E1T = exp(S^T/8)                  [ACT, PSUM->SBUF bf16]
    rowsum1 (over k = partitions) via ones[128,128]-stationary matmul ->
      replicated [128,1024] PSUM; r1 = recip_approx_fast (f32) -> bf16
    D = E1T * r1 (broadcast along kb)  [DVE TT bf16, in place]
    E3T = exp(D) [ACT]; E2T = 1/E3T = exp(-D) [DVE recip_approx bf16]
    O3^T/O2^T accumulate via V_aug-stationary (65 rows incl ones col)
    evac bf16 -> dma_start_transpose -> [tok, 65]; divide by col 64 [DVE].
"""

import sys

if "/opt/trn_rl_repo" not in sys.path:
    sys.path.insert(0, "/opt/trn_rl_repo")

from contextlib import ExitStack

import numpy as np

import concourse.bass as bass
from concourse import bacc, mybir
from concourse.bass_utils import run_bass_kernel_spmd
from concourse.dve_ops import RECIP_APPROX_FAST_CONSTS, RECIPROCAL_APPROX_FAST
from concourse.tile import TileContext

F32 = mybir.dt.float32
BF = mybir.dt.bfloat16
EXP = mybir.ActivationFunctionType.Exp
ADD = mybir.AluOpType.add
MUL = mybir.AluOpType.mult

P = 128          # partitions
N = 1024         # tokens
D = 1024         # model dim
NF = 512         # features per core (8 heads x 64)
FH = 8           # heads per core
DK = 64          # head dim
NPT = N // P     # 8 token ptiles
KBN = D // P     # 8 contraction blocks
MB = NF // P     # 4 feature ptiles
HB = KBN // 2    # kb half-batch for elementwise ops

# how many of the 2 per-head [128,4,1024] E2 batches go to ACT exp(-D)
# instead of DVE reciprocal-of-E3 (tune for ACT/DVE balance)
E2_ACT_BATCHES = 1


def build_nc():
    nc = bacc.Bacc("TRN2")
    x_d = nc.dram_tensor("x", [N, D], F32, kind="ExternalInput")
    y_d = nc.dram_tensor("y", [N, D], F32, kind="ExternalInput")
    wq_d = nc.dram_tensor("wq", [NF, D], F32, kind="ExternalInput")
    wk_d = nc.dram_tensor("wk", [NF, D], F32, kind="ExternalInput")
    wv_d = nc.dram_tensor("wv", [NF, D], F32, kind="ExternalInput")
    catt_d = nc.dram_tensor("catt", [N, NF], BF, kind="ExternalOutput")
    att_d = nc.dram_tensor("att", [N, NF], BF, kind="ExternalOutput")

    with TileContext(nc) as tc, ExitStack() as ctx:
        persist = ctx.enter_context(tc.tile_pool(name="persist", bufs=1))
        qt = persist.tile([P, MB, N], BF)        # Q^T: [feat%128, featblk, tok]
        kt = persist.tile([P, MB, N], BF)
        vv = persist.tile([P, NPT, FH, DK + 1], BF)   # V_aug per head
        att_sb = persist.tile([P, NPT, NF], BF)
        catt_sb = persist.tile([P, NPT, NF], BF)
        ones_bf = persist.tile([P, P], BF)
        nc.vector.memset(ones_bf[:], 1.0)
        nc.vector.memset(vv[:, :, :, DK:DK + 1], 1.0)

        # pools phase1 needs (created early so phase1 of the first heads can
        # interleave with the setup projections)
        e1p = ctx.enter_context(tc.tile_pool(name="e1p", bufs=2))
        rsp = ctx.enter_context(tc.tile_pool(name="rsp", bufs=2))
        psb = ctx.enter_context(tc.tile_pool(name="psb", bufs=2, space="PSUM"))
        rsb = ctx.enter_context(tc.tile_pool(name="rsb", bufs=1, space="PSUM"))

        cc = RECIP_APPROX_FAST_CONSTS

        def phase1a(h):
            """S^T matmuls + E1T exp."""
            hb, ho = h // 2, (h % 2) * DK
            e1t = e1p.tile([P, KBN, N], BF, tag="e1", name=f"e1t_{h}")
            for kb in range(KBN):
                s_ps = psb.tile([P, N], F32, tag="st", name=f"s_{h}_{kb}")
                for ch in range(2):
                    sl = slice(ch * 512, (ch + 1) * 512)
                    nc.tensor.matmul(
                        s_ps[:, sl],
                        lhsT=kt[ho:ho + DK, hb, kb * P:(kb + 1) * P],
                        rhs=qt[ho:ho + DK, hb, sl],
                        start=True,
                        stop=True,
                    )
                nc.scalar.activation(e1t[:, kb, :], s_ps[:], EXP,
                                     scale=0.125)
            return e1t

        def phase1b(h, e1t):
            """rowsum over k (partitions) + 1/rowsum."""
            rs_ps = rsb.tile([P, N], F32, tag="rs", name=f"rs_{h}")
            for kb in range(KBN):
                for ch in range(2):
                    sl = slice(ch * 512, (ch + 1) * 512)
                    nc.tensor.matmul(
                        rs_ps[:, sl],
                        lhsT=ones_bf[:],
                        rhs=e1t[:, kb, sl],
                        start=(kb == 0),
                        stop=(kb == KBN - 1),
                    )
            r1b = rsp.tile([P, N], BF, tag="r1b", name=f"r1b_{h}")
            nc.vector._custom_dve(
                RECIPROCAL_APPROX_FAST, out=r1b[:], in0=rs_ps[:],
                s0=cc["s0"], s1=cc["s1"], imm2=cc["imm2"])
            return e1t, r1b

        def phase1(h):
            return phase1b(h, phase1a(h))

        # ---------------- setup: transposed loads + projections ----------------
        states = {}
        with ExitStack() as sctx:
            sbp = sctx.enter_context(tc.tile_pool(name="setup", bufs=1))
            pst = sctx.enter_context(tc.tile_pool(name="pst", bufs=2, space="PSUM"))

            xt = sbp.tile([P, KBN, N], BF, tag="xt")
            yt = sbp.tile([P, KBN, N], BF, tag="yt")
            wqt = sbp.tile([P, KBN, NF], BF, tag="wqt")
            wkt = sbp.tile([P, KBN, NF], BF, tag="wkt")
            wvt = sbp.tile([P, KBN, NF], BF, tag="wvt")

            def proj_qk(m):
                for wt, out_sb in ((wqt, qt), (wkt, kt)):
                    for ch in range(2):
                        sl = slice(ch * 512, (ch + 1) * 512)
                        ps = pst.tile([P, 512], F32, tag="proj")
                        for kb in range(KBN):
                            nc.tensor.matmul(
                                ps[:],
                                lhsT=wt[:, kb, m * P:(m + 1) * P],
                                rhs=xt[:, kb, sl],
                                start=(kb == 0),
                                stop=(kb == KBN - 1),
                            )
                        nc.vector.tensor_copy(out_sb[:, m, sl], ps[:])

            # All loads first, then all transposes (the scheduler serializes
            # every xbar mode switch, so they must not interleave). Early
            # heads' phase 1 and the projections fill the PE/ACT meanwhile.
            mats = ((x_d, NPT, xt), (wq_d, MB, wqt), (wk_d, MB, wkt),
                    (y_d, NPT, yt), (wv_d, MB, wvt))
            with ExitStack() as lctx:
                raw = lctx.enter_context(tc.tile_pool(name="raw", bufs=4))
                bfp = lctx.enter_context(tc.tile_pool(name="bfp", bufs=1))
                slabs = []
                for mi, (src_d, nslab, dst) in enumerate(mats):
                    bft = bfp.tile([P, nslab, D], BF, tag=f"bf{mi}",
                                   name=f"bf{mi}")
                    slabs.append(bft)
                    for s in range(nslab):
                        rw = raw.tile([P, D], F32, tag="raw")
                        nc.sync.dma_start(
                            out=rw[:], in_=src_d[s * P:(s + 1) * P, :])
                        nc.vector.tensor_copy(bft[:, s, :], rw[:])
                tc.no_sync_barrier()

                def transposes(lo, hi):
                    for mi in range(lo, hi):
                        _, nslab, dst = mats[mi]
                        for s in range(nslab):
                            nc.scalar.dma_start_transpose(
                                out=dst[:, :, s * P:(s + 1) * P],
                                in_=slabs[mi][:, s, :])

                transposes(0, 3)          # x, wq, wk
                proj_qk(0)
                e1t0 = phase1a(0)
                e1t1 = phase1a(1)
                transposes(3, 5)          # y, wv
                proj_qk(1)
                states[0] = phase1b(0, e1t0)
                proj_qk(2)
                states[1] = phase1b(1, e1t1)
                proj_qk(3)
            for i in range(NPT):
                ps = pst.tile([P, 512], F32, tag="proj")
                for kb in range(KBN):
                    nc.tensor.matmul(
                        ps[:],
                        lhsT=yt[:, kb, i * P:(i + 1) * P],
                        rhs=wvt[:, kb, :],
                        start=(kb == 0),
                        stop=(kb == KBN - 1),
                    )
                nc.vector.tensor_copy(
                    vv[:, i, :, 0:DK],
                    ps[:].rearrange("p (h d) -> p h d", h=FH),
                )

        # ---------------- per-head attention ----------------
        e3p = ctx.enter_context(tc.tile_pool(name="e3p", bufs=4))
        e2p = ctx.enter_context(tc.tile_pool(name="e2p", bufs=4))
        osbp = ctx.enter_context(tc.tile_pool(name="osbp", bufs=4))
        otsp = ctx.enter_context(tc.tile_pool(name="otsp", bufs=4))
        smp = ctx.enter_context(tc.tile_pool(name="smp", bufs=24))
        pso = ctx.enter_context(tc.tile_pool(name="pso", bufs=2, space="PSUM"))

        def phase2(h, e1t, r1b):
            """dist mul, second exps, O matmuls, transpose-out, normalize."""
            r1x = r1b.rearrange("p (o n) -> p o n", o=1).broadcast_to([P, 2, N])
            e3s, e2s = [], []
            for half in range(4):
                d2 = e1t[:, half * 2:(half + 1) * 2, :]
                # D = dist (in place over E1T)
                nc.vector.tensor_mul(d2, d2, r1x)
                e3 = e3p.tile([P, 2, N], BF, tag="e3", name=f"e3_{h}_{half}")
                nc.scalar.activation(e3[:], d2, EXP)
                e2 = e2p.tile([P, 2, N], BF, tag="e2", name=f"e2_{h}_{half}")
                if half < E2_ACT_BATCHES:
                    nc.scalar.activation(e2[:], d2, EXP, scale=-1.0)
                else:
                    nc.vector._custom_dve(
                        RECIPROCAL_APPROX_FAST, out=e2[:], in0=e3[:],
                        s0=cc["s0"], s1=cc["s1"], imm2=cc["imm2"])
                e3s.append(e3)
                e2s.append(e2)

            osb3 = osbp.tile([80, N], BF, tag="osb", name=f"osb3_{h}")
            osb2 = osbp.tile([80, N], BF, tag="osb", name=f"osb2_{h}")
            for ch in range(2):
                sl = slice(ch * 512, (ch + 1) * 512)
                o3_ps = pso.tile([DK + 1, 512], F32, tag="o", name=f"o3_{h}_{ch}")
                o2_ps = pso.tile([DK + 1, 512], F32, tag="o", name=f"o2_{h}_{ch}")
                for kb in range(KBN):
                    nc.tensor.matmul(
                        o3_ps[:], lhsT=vv[:, kb, h, :],
                        rhs=e3s[kb // 2][:, kb % 2, sl],
                        start=(kb == 0), stop=(kb == KBN - 1),
                    )
                    nc.tensor.matmul(
                        o2_ps[:], lhsT=vv[:, kb, h, :],
                        rhs=e2s[kb // 2][:, kb % 2, sl],
                        start=(kb == 0), stop=(kb == KBN - 1),
                    )
                nc.vector.tensor_copy(osb3[0:DK + 1, sl], o3_ps[:])
                nc.vector.tensor_copy(osb2[0:DK + 1, sl], o2_ps[:])

            for osb_t, out_t in ((osb3, att_sb), (osb2, catt_sb)):
                ot = otsp.tile([P, NPT, 80], BF, tag="ots", name=f"ot_{h}")
                nc.sync.dma_start_transpose(out=ot[:], in_=osb_t[:])
                # one batched reciprocal of the 8 rowsums, then per-tile
                # scalar multiplies; these are leaf ops, so half go to the
                # otherwise-idle gpsimd engine
                rr = smp.tile([P, NPT], F32, tag="rr", name=f"rr_{h}")
                nc.vector.reciprocal(rr[:], ot[:, :, DK])
                for i in range(NPT):
                    eng = nc.gpsimd if i % 2 == 0 else nc.vector
                    eng.tensor_scalar_mul(
                        out_t[:, i, h * DK:(h + 1) * DK], ot[:, i, 0:DK],
                        rr[:, i:i + 1])

        # software pipeline (heads 0-1 already ran phase 1 during setup):
        # phase2(h) is emitted just after phase1(h+2)'s work is in flight
        for h in range(2, FH):
            phase2(h - 2, *states.pop(h - 2))
            states[h] = phase1(h)
        phase2(FH - 2, *states.pop(FH - 2))
        phase2(FH - 1, *states.pop(FH - 1))

        for i in range(NPT):
            nc.sync.dma_start(out=att_d[i * P:(i + 1) * P, :], in_=att_sb[:, i, :])
            nc.sync.dma_start(out=catt_d[i * P:(i + 1) * P, :], in_=catt_sb[:, i, :])

    nc.finalize()
    return nc


_NC_CACHE = {}


def _get_nc():
    if "nc" not in _NC_CACHE:
        _NC_CACHE["nc"] = build_nc()
    return _NC_CACHE["nc"]


def _make_in_maps(x, y, Wq, Wk, Wv):
    x = np.ascontiguousarray(np.asarray(x, dtype=np.float32))
    y = np.ascontiguousarray(np.asarray(y, dtype=np.float32))
    Wq = np.ascontiguousarray(np.asarray(Wq, dtype=np.float32))
    Wk = np.ascontiguousarray(np.asarray(Wk, dtype=np.float32))
    Wv = np.ascontiguousarray(np.asarray(Wv, dtype=np.float32))
    in_maps = []
    for c in range(8):
        b, h0 = c // 2, (c % 2) * 8
        rows = slice(h0 * DK, h0 * DK + NF)
        in_maps.append({
            "x": x[b],
            "y": y[b],
            "wq": np.ascontiguousarray(Wq[rows]),
            "wk": np.ascontiguousarray(Wk[rows]),
            "wv": np.ascontiguousarray(Wv[rows]),
        })
    return in_maps


def run_cores(x, y, Wq, Wk, Wv, trace=False, tmpdir=None):
    nc = _get_nc()
    res = run_bass_kernel_spmd(
        nc, _make_in_maps(x, y, Wq, Wk, Wv), core_ids=list(range(8)),
        trace=trace, tmpdir=tmpdir,
    )
    B = 4
    c_att = np.empty((B, N, 2 * NF), dtype=np.float32)
    att = np.empty((B, N, 2 * NF), dtype=np.float32)
    for c, r in enumerate(res.results):
        b, cols = c // 2, slice((c % 2) * NF, (c % 2) * NF + NF)
        c_att[b][:, cols] = np.asarray(r["catt"], dtype=np.float32)
        att[b][:, cols] = np.asarray(r["att"], dtype=np.float32)
    return (c_att, att), res


def kernel(x, y, Wq, Wk, Wv):
    out, _ = run_cores(x, y, Wq, Wk, Wv)
    return out


# revision 27
# speedup vs baseline: 1.0715x; 1.0715x over previous
"""Trainium2 Bass kernel: dual-softmax ("contrast") multi-head self-attention.

Problem (per full input):
  x, y: (4, 1024, 1024) f32; Wq/Wk/Wv: (1024, 1024) f32, nh=16 heads, dk=dv=64.
  q = x @ Wq.T, k = x @ Wk.T, v = y @ Wv.T  (split heads)
  dist   = softmax(q k^T / 8)
  c_att  = softmax(1 - dist) @ v      (== softmax(-dist) @ v, shift invariance)
  att    = softmax(dist) @ v
  returns (c_att, att), each (4, 1024, 1024) f32.

Sharding: 8 cores = 4 batches x 2 head-groups (8 heads each). Each core gets
x[b], y[b] and a 512-row slice of each weight; returns (c_att, att) slices
[1024, 512].

V2 design (bf16 matmuls, transpose-free scores):
  Loads arrive pre-transposed: DMA f32 -> DVE cast bf16 -> SBUF->SBUF
  dma_start_transpose (xbar) -> xt/yt/wqt/wkt/wvt in [contract-dim, free] form.
  QT = wqt.T @ xt, KT = wkt.T @ xt   [feat, tok] bf16; V via lhsT=yt slices.
  Per head, k-major throughout (no PE transposes of the score matrix):
    S^T[k,q] = KT_h^T QT_h via matmul (16 x 512-col bf16 MMs)
    E1T = exp(S^T/8)                  [ACT, PSUM->SBUF bf16]
    rowsum1 (over k = partitions) via ones[128,128]-stationary matmul ->
      replicated [128,1024] PSUM; r1 = recip_approx_fast (f32) -> bf16
    D = E1T * r1 (broadcast along kb)  [DVE TT bf16, in place]
    E3T = exp(D) [ACT]; E2T = 1/E3T = exp(-D) [DVE recip_approx bf16]
    O3^T/O2^T accumulate via V_aug-stationary (65 rows incl ones col)
    evac bf16 -> dma_start_transpose -> [tok, 65]; divide by col 64 [DVE].
"""

import sys

if "/opt/trn_rl_repo" not in sys.path:
    sys.path.insert(0, "/opt/trn_rl_repo")

from contextlib import ExitStack

import numpy as np

import concourse.bass as bass
from concourse import bacc, mybir
from concourse.bass_utils import run_bass_kernel_spmd
from concourse.dve_ops import RECIP_APPROX_FAST_CONSTS, RECIPROCAL_APPROX_FAST
from concourse.tile import TileContext

F32 = mybir.dt.float32
BF = mybir.dt.bfloat16
EXP = mybir.ActivationFunctionType.Exp
ADD = mybir.AluOpType.add
MUL = mybir.AluOpType.mult

P = 128          # partitions
N = 1024         # tokens
D = 1024         # model dim
NF = 512         # features per core (8 heads x 64)
FH = 8           # heads per core
DK = 64          # head dim
NPT = N // P     # 8 token ptiles
KBN = D // P     # 8 contraction blocks
MB = NF // P     # 4 feature ptiles
HB = KBN // 2    # kb half-batch for elementwise ops

# how many of the 2 per-head [128,4,1024] E2 batches go to ACT exp(-D)
# instead of DVE reciprocal-of-E3 (tune for ACT/DVE balance)
E2_ACT_BATCHES = 1


def build_nc():
    nc = bacc.Bacc("TRN2")
    x_d = nc.dram_tensor("x", [N, D], F32, kind="ExternalInput")
    y_d = nc.dram_tensor("y", [N, D], F32, kind="ExternalInput")
    wq_d = nc.dram_tensor("wq", [NF, D], F32, kind="ExternalInput")
    wk_d = nc.dram_tensor("wk", [NF, D], F32, kind="ExternalInput")
    wv_d = nc.dram_tensor("wv", [NF, D], F32, kind="ExternalInput")
    catt_d = nc.dram_tensor("catt", [N, NF], BF, kind="ExternalOutput")
    att_d = nc.dram_tensor("att", [N, NF], BF, kind="ExternalOutput")

    with TileContext(nc) as tc, ExitStack() as ctx:
        persist = ctx.enter_context(tc.tile_pool(name="persist", bufs=1))
        qt = persist.tile([P, MB, N], BF)        # Q^T: [feat%128, featblk, tok]
        kt = persist.tile([P, MB, N], BF)
        vv = persist.tile([P, NPT, FH, DK + 1], BF)   # V_aug per head
        att_sb = persist.tile([P, NPT, NF], BF)
        catt_sb = persist.tile([P, NPT, NF], BF)
        ones_bf = persist.tile([P, P], BF)
        nc.vector.memset(ones_bf[:], 1.0)
        nc.vector.memset(vv[:, :, :, DK:DK + 1], 1.0)

        # pools phase1 needs (created early so phase1 of the first heads can
        # interleave with the setup projections)
        e1p = ctx.enter_context(tc.tile_pool(name="e1p", bufs=2))
        rsp = ctx.enter_context(tc.tile_pool(name="rsp", bufs=2))
        psb = ctx.enter_context(tc.tile_pool(name="psb", bufs=2, space="PSUM"))
        rsb = ctx.enter_context(tc.tile_pool(name="rsb", bufs=1, space="PSUM"))

        cc = RECIP_APPROX_FAST_CONSTS

        def phase1a(h):
            """S^T matmuls + E1T exp."""
            hb, ho = h // 2, (h % 2) * DK
            e1t = e1p.tile([P, KBN, N], BF, tag="e1", name=f"e1t_{h}")
            for kb in range(KBN):
                s_ps = psb.tile([P, N], F32, tag="st", name=f"s_{h}_{kb}")
                for ch in range(2):
                    sl = slice(ch * 512, (ch + 1) * 512)
                    nc.tensor.matmul(
                        s_ps[:, sl],
                        lhsT=kt[ho:ho + DK, hb, kb * P:(kb + 1) * P],
                        rhs=qt[ho:ho + DK, hb, sl],
                        start=True,
                        stop=True,
                    )
                nc.scalar.activation(e1t[:, kb, :], s_ps[:], EXP,
                                     scale=0.125)
            return e1t

        def phase1b(h, e1t):
            """rowsum over k (partitions) + 1/rowsum."""
            rs_ps = rsb.tile([P, N], F32, tag="rs", name=f"rs_{h}")
            for kb in range(KBN):
                for ch in range(2):
                    sl = slice(ch * 512, (ch + 1) * 512)
                    nc.tensor.matmul(
                        rs_ps[:, sl],
                        lhsT=ones_bf[:],
                        rhs=e1t[:, kb, sl],
                        start=(kb == 0),
                        stop=(kb == KBN - 1),
                    )
            r1b = rsp.tile([P, N], BF, tag="r1b", name=f"r1b_{h}")
            nc.vector._custom_dve(
                RECIPROCAL_APPROX_FAST, out=r1b[:], in0=rs_ps[:],
                s0=cc["s0"], s1=cc["s1"], imm2=cc["imm2"])
            return e1t, r1b

        def phase1(h):
            return phase1b(h, phase1a(h))

        # ---------------- setup: transposed loads + projections ----------------
        states = {}
        with ExitStack() as sctx:
            sbp = sctx.enter_context(tc.tile_pool(name="setup", bufs=1))
            pst = sctx.enter_context(tc.tile_pool(name="pst", bufs=2, space="PSUM"))

            xt = sbp.tile([P, KBN, N], BF, tag="xt")
            yt = sbp.tile([P, KBN, N], BF, tag="yt")
            wqt = sbp.tile([P, KBN, NF], BF, tag="wqt")
            wkt = sbp.tile([P, KBN, NF], BF, tag="wkt")
            wvt = sbp.tile([P, KBN, NF], BF, tag="wvt")

            def proj_qk(m):
                for wt, out_sb in ((wqt, qt), (wkt, kt)):
                    for ch in range(2):
                        sl = slice(ch * 512, (ch + 1) * 512)
                        ps = pst.tile([P, 512], F32, tag="proj")
                        for kb in range(KBN):
                            nc.tensor.matmul(
                                ps[:],
                                lhsT=wt[:, kb, m * P:(m + 1) * P],
                                rhs=xt[:, kb, sl],
                                start=(kb == 0),
                                stop=(kb == KBN - 1),
                            )
                        nc.vector.tensor_copy(out_sb[:, m, sl], ps[:])

            # Loads cast f32->bf16 in-flight via SWDGE (gpsimd DMA): no f32
            # staging, no DVE cast pass. Loads and xbar transposes must not
            # interleave (the scheduler serializes every xbar mode switch),
            # so they are fenced into two groups. Early heads' phase 1 and
            # the projections fill the PE/ACT meanwhile.
            mats = ((x_d, NPT, xt), (wq_d, MB, wqt), (wk_d, MB, wkt),
                    (y_d, NPT, yt), (wv_d, MB, wvt))
            with ExitStack() as lctx:
                bfp = lctx.enter_context(tc.tile_pool(name="bfp", bufs=1))
                slabs = []
                for mi, (src_d, nslab, dst) in enumerate(mats):
                    bft = bfp.tile([P, nslab, D], BF, tag=f"bf{mi}",
                                   name=f"bf{mi}")
                    slabs.append(bft)

                def cast_loads(lo, hi):
                    for mi in range(lo, hi):
                        src_d, nslab, _ = mats[mi]
                        for s in range(nslab):
                            nc.gpsimd.dma_start(
                                out=slabs[mi][:, s, :],
                                in_=src_d[s * P:(s + 1) * P, :])

                def transposes(lo, hi):
                    for mi in range(lo, hi):
                        _, nslab, dst = mats[mi]
                        for s in range(nslab):
                            nc.scalar.dma_start_transpose(
                                out=dst[:, :, s * P:(s + 1) * P],
                                in_=slabs[mi][:, s, :])

                cast_loads(0, 3)          # x, wq, wk
                tc.no_sync_barrier()
                transposes(0, 3)
                tc.no_sync_barrier()
                cast_loads(3, 5)          # y, wv
                proj_qk(0)
                e1t0 = phase1a(0)
                e1t1 = phase1a(1)
                tc.no_sync_barrier()
                transposes(3, 5)
                proj_qk(1)
                states[0] = phase1b(0, e1t0)
                proj_qk(2)
                states[1] = phase1b(1, e1t1)
                proj_qk(3)
            for i in range(NPT):
                ps = pst.tile([P, 512], F32, tag="proj")
                for kb in range(KBN):
                    nc.tensor.matmul(
                        ps[:],
                        lhsT=yt[:, kb, i * P:(i + 1) * P],
                        rhs=wvt[:, kb, :],
                        start=(kb == 0),
                        stop=(kb == KBN - 1),
                    )
                nc.vector.tensor_copy(
                    vv[:, i, :, 0:DK],
                    ps[:].rearrange("p (h d) -> p h d", h=FH),
                )

        # ---------------- per-head attention ----------------
        e3p = ctx.enter_context(tc.tile_pool(name="e3p", bufs=4))
        e2p = ctx.enter_context(tc.tile_pool(name="e2p", bufs=4))
        osbp = ctx.enter_context(tc.tile_pool(name="osbp", bufs=4))
        otsp = ctx.enter_context(tc.tile_pool(name="otsp", bufs=4))
        smp = ctx.enter_context(tc.tile_pool(name="smp", bufs=24))
        pso = ctx.enter_context(tc.tile_pool(name="pso", bufs=2, space="PSUM"))

        def phase2(h, e1t, r1b):
            """dist mul, second exps, O matmuls, transpose-out, normalize."""
            r1x = r1b.rearrange("p (o n) -> p o n", o=1).broadcast_to([P, 2, N])
            e3s, e2s = [], []
            for half in range(4):
                d2 = e1t[:, half * 2:(half + 1) * 2, :]
                # D = dist (in place over E1T)
                nc.vector.tensor_mul(d2, d2, r1x)
                e3 = e3p.tile([P, 2, N], BF, tag="e3", name=f"e3_{h}_{half}")
                nc.scalar.activation(e3[:], d2, EXP)
                e2 = e2p.tile([P, 2, N], BF, tag="e2", name=f"e2_{h}_{half}")
                if half < E2_ACT_BATCHES:
                    nc.scalar.activation(e2[:], d2, EXP, scale=-1.0)
                else:
                    nc.vector._custom_dve(
                        RECIPROCAL_APPROX_FAST, out=e2[:], in0=e3[:],
                        s0=cc["s0"], s1=cc["s1"], imm2=cc["imm2"])
                e3s.append(e3)
                e2s.append(e2)

            osb3 = osbp.tile([80, N], BF, tag="osb", name=f"osb3_{h}")
            osb2 = osbp.tile([80, N], BF, tag="osb", name=f"osb2_{h}")
            for ch in range(2):
                sl = slice(ch * 512, (ch + 1) * 512)
                o3_ps = pso.tile([DK + 1, 512], F32, tag="o", name=f"o3_{h}_{ch}")
                o2_ps = pso.tile([DK + 1, 512], F32, tag="o", name=f"o2_{h}_{ch}")
                for kb in range(KBN):
                    nc.tensor.matmul(
                        o3_ps[:], lhsT=vv[:, kb, h, :],
                        rhs=e3s[kb // 2][:, kb % 2, sl],
                        start=(kb == 0), stop=(kb == KBN - 1),
                    )
                    nc.tensor.matmul(
                        o2_ps[:], lhsT=vv[:, kb, h, :],
                        rhs=e2s[kb // 2][:, kb % 2, sl],
                        start=(kb == 0), stop=(kb == KBN - 1),
                    )
                nc.vector.tensor_copy(osb3[0:DK + 1, sl], o3_ps[:])
                nc.vector.tensor_copy(osb2[0:DK + 1, sl], o2_ps[:])

            for osb_t, out_t in ((osb3, att_sb), (osb2, catt_sb)):
                ot = otsp.tile([P, NPT, 80], BF, tag="ots", name=f"ot_{h}")
                nc.sync.dma_start_transpose(out=ot[:], in_=osb_t[:])
                # one batched reciprocal of the 8 rowsums, then per-tile
                # scalar multiplies; these are leaf ops, so half go to the
                # otherwise-idle gpsimd engine
                rr = smp.tile([P, NPT], F32, tag="rr", name=f"rr_{h}")
                nc.vector.reciprocal(rr[:], ot[:, :, DK])
                for i in range(NPT):
                    eng = nc.gpsimd if i % 2 == 0 else nc.vector
                    eng.tensor_scalar_mul(
                        out_t[:, i, h * DK:(h + 1) * DK], ot[:, i, 0:DK],
                        rr[:, i:i + 1])

        # software pipeline (heads 0-1 already ran phase 1 during setup):
        # phase2(h) is emitted just after phase1(h+2)'s work is in flight
        for h in range(2, FH):
            phase2(h - 2, *states.pop(h - 2))
            states[h] = phase1(h)
        phase2(FH - 2, *states.pop(FH - 2))
        phase2(FH - 1, *states.pop(FH - 1))

        for i in range(NPT):
            nc.sync.dma_start(out=att_d[i * P:(i + 1) * P, :], in_=att_sb[:, i, :])
            nc.sync.dma_start(out=catt_d[i * P:(i + 1) * P, :], in_=catt_sb[:, i, :])

    nc.finalize()
    return nc


_NC_CACHE = {}


def _get_nc():
    if "nc" not in _NC_CACHE:
        _NC_CACHE["nc"] = build_nc()
    return _NC_CACHE["nc"]


def _make_in_maps(x, y, Wq, Wk, Wv):
    x = np.ascontiguousarray(np.asarray(x, dtype=np.float32))
    y = np.ascontiguousarray(np.asarray(y, dtype=np.float32))
    Wq = np.ascontiguousarray(np.asarray(Wq, dtype=np.float32))
    Wk = np.ascontiguousarray(np.asarray(Wk, dtype=np.float32))
    Wv = np.ascontiguousarray(np.asarray(Wv, dtype=np.float32))
    in_maps = []
    for c in range(8):
        b, h0 = c // 2, (c % 2) * 8
        rows = slice(h0 * DK, h0 * DK + NF)
        in_maps.append({
            "x": x[b],
            "y": y[b],
            "wq": np.ascontiguousarray(Wq[rows]),
            "wk": np.ascontiguousarray(Wk[rows]),
            "wv": np.ascontiguousarray(Wv[rows]),
        })
    return in_maps


def run_cores(x, y, Wq, Wk, Wv, trace=False, tmpdir=None):
    nc = _get_nc()
    res = run_bass_kernel_spmd(
        nc, _make_in_maps(x, y, Wq, Wk, Wv), core_ids=list(range(8)),
        trace=trace, tmpdir=tmpdir,
    )
    B = 4
    c_att = np.empty((B, N, 2 * NF), dtype=np.float32)
    att = np.empty((B, N, 2 * NF), dtype=np.float32)
    for c, r in enumerate(res.results):
        b, cols = c // 2, slice((c % 2) * NF, (c % 2) * NF + NF)
        c_att[b][:, cols] = np.asarray(r["catt"], dtype=np.float32)
        att[b][:, cols] = np.asarray(r["att"], dtype=np.float32)
    return (c_att, att), res


def kernel(x, y, Wq, Wk, Wv):
    out, _ = run_cores(x, y, Wq, Wk, Wv)
    return out


# revision 28
# speedup vs baseline: 1.0799x; 1.0078x over previous
"""Trainium2 Bass kernel: dual-softmax ("contrast") multi-head self-attention.

Problem (per full input):
  x, y: (4, 1024, 1024) f32; Wq/Wk/Wv: (1024, 1024) f32, nh=16 heads, dk=dv=64.
  q = x @ Wq.T, k = x @ Wk.T, v = y @ Wv.T  (split heads)
  dist   = softmax(q k^T / 8)
  c_att  = softmax(1 - dist) @ v      (== softmax(-dist) @ v, shift invariance)
  att    = softmax(dist) @ v
  returns (c_att, att), each (4, 1024, 1024) f32.

Sharding: 8 cores = 4 batches x 2 head-groups (8 heads each). Each core gets
x[b], y[b] and a 512-row slice of each weight; returns (c_att, att) slices
[1024, 512].

V2 design (bf16 matmuls, transpose-free scores):
  Loads arrive pre-transposed: DMA f32 -> DVE cast bf16 -> SBUF->SBUF
  dma_start_transpose (xbar) -> xt/yt/wqt/wkt/wvt in [contract-dim, free] form.
  QT = wqt.T @ xt, KT = wkt.T @ xt   [feat, tok] bf16; V via lhsT=yt slices.
  Per head, k-major throughout (no PE transposes of the score matrix):
    S^T[k,q] = KT_h^T QT_h via matmul (16 x 512-col bf16 MMs)
    E1T = exp(S^T/8)                  [ACT, PSUM->SBUF bf16]
    rowsum1 (over k = partitions) via ones[128,128]-stationary matmul ->
      replicated [128,1024] PSUM; r1 = recip_approx_fast (f32) -> bf16
    D = E1T * r1 (broadcast along kb)  [DVE TT bf16, in place]
    E3T = exp(D) [ACT]; E2T = 1/E3T = exp(-D) [DVE recip_approx bf16]
    O3^T/O2^T accumulate via V_aug-stationary (65 rows incl ones col)
    evac bf16 -> dma_start_transpose -> [tok, 65]; divide by col 64 [DVE].
"""

import sys

if "/opt/trn_rl_repo" not in sys.path:
    sys.path.insert(0, "/opt/trn_rl_repo")

from contextlib import ExitStack

import numpy as np

import concourse.bass as bass
from concourse import bacc, mybir
from concourse.bass_utils import run_bass_kernel_spmd
from concourse.dve_ops import RECIP_APPROX_FAST_CONSTS, RECIPROCAL_APPROX_FAST
from concourse.tile import TileContext

F32 = mybir.dt.float32
BF = mybir.dt.bfloat16
EXP = mybir.ActivationFunctionType.Exp
ADD = mybir.AluOpType.add
MUL = mybir.AluOpType.mult

P = 128          # partitions
N = 1024         # tokens
D = 1024         # model dim
NF = 512         # features per core (8 heads x 64)
FH = 8           # heads per core
DK = 64          # head dim
NPT = N // P     # 8 token ptiles
KBN = D // P     # 8 contraction blocks
MB = NF // P     # 4 feature ptiles
HB = KBN // 2    # kb half-batch for elementwise ops

# how many of the 2 per-head [128,4,1024] E2 batches go to ACT exp(-D)
# instead of DVE reciprocal-of-E3 (tune for ACT/DVE balance)
E2_ACT_BATCHES = 1


def build_nc():
    nc = bacc.Bacc("TRN2")
    x_d = nc.dram_tensor("x", [N, D], F32, kind="ExternalInput")
    y_d = nc.dram_tensor("y", [N, D], F32, kind="ExternalInput")
    wq_d = nc.dram_tensor("wq", [NF, D], F32, kind="ExternalInput")
    wk_d = nc.dram_tensor("wk", [NF, D], F32, kind="ExternalInput")
    wv_d = nc.dram_tensor("wv", [NF, D], F32, kind="ExternalInput")
    catt_d = nc.dram_tensor("catt", [N, NF], BF, kind="ExternalOutput")
    att_d = nc.dram_tensor("att", [N, NF], BF, kind="ExternalOutput")

    with TileContext(nc) as tc, ExitStack() as ctx:
        persist = ctx.enter_context(tc.tile_pool(name="persist", bufs=1))
        qt = persist.tile([P, MB, N], BF)        # Q^T: [feat%128, featblk, tok]
        kt = persist.tile([P, MB, N], BF)
        vv = persist.tile([P, NPT, FH, DK + 1], BF)   # V_aug per head
        att_sb = persist.tile([P, NPT, NF], BF)
        catt_sb = persist.tile([P, NPT, NF], BF)
        ones_bf = persist.tile([P, P], BF)
        nc.vector.memset(ones_bf[:], 1.0)
        nc.vector.memset(vv[:, :, :, DK:DK + 1], 1.0)

        # pools phase1 needs (created early so phase1 of the first heads can
        # interleave with the setup projections)
        e1p = ctx.enter_context(tc.tile_pool(name="e1p", bufs=2))
        rsp = ctx.enter_context(tc.tile_pool(name="rsp", bufs=2))
        psb = ctx.enter_context(tc.tile_pool(name="psb", bufs=2, space="PSUM"))
        rsb = ctx.enter_context(tc.tile_pool(name="rsb", bufs=1, space="PSUM"))

        cc = RECIP_APPROX_FAST_CONSTS

        def phase1a(h):
            """S^T matmuls + E1T exp."""
            hb, ho = h // 2, (h % 2) * DK
            e1t = e1p.tile([P, KBN, N], BF, tag="e1", name=f"e1t_{h}")
            for kb in range(KBN):
                s_ps = psb.tile([P, N], F32, tag="st", name=f"s_{h}_{kb}")
                for ch in range(2):
                    sl = slice(ch * 512, (ch + 1) * 512)
                    nc.tensor.matmul(
                        s_ps[:, sl],
                        lhsT=kt[ho:ho + DK, hb, kb * P:(kb + 1) * P],
                        rhs=qt[ho:ho + DK, hb, sl],
                        start=True,
                        stop=True,
                    )
                nc.scalar.activation(e1t[:, kb, :], s_ps[:], EXP,
                                     scale=0.125)
            return e1t

        def phase1b(h, e1t):
            """rowsum over k (partitions) + 1/rowsum."""
            rs_ps = rsb.tile([P, N], F32, tag="rs", name=f"rs_{h}")
            for kb in range(KBN):
                for ch in range(2):
                    sl = slice(ch * 512, (ch + 1) * 512)
                    nc.tensor.matmul(
                        rs_ps[:, sl],
                        lhsT=ones_bf[:],
                        rhs=e1t[:, kb, sl],
                        start=(kb == 0),
                        stop=(kb == KBN - 1),
                    )
            r1b = rsp.tile([P, N], BF, tag="r1b", name=f"r1b_{h}")
            nc.vector._custom_dve(
                RECIPROCAL_APPROX_FAST, out=r1b[:], in0=rs_ps[:],
                s0=cc["s0"], s1=cc["s1"], imm2=cc["imm2"])
            return e1t, r1b

        def phase1(h):
            return phase1b(h, phase1a(h))

        # ---------------- setup: transposed loads + projections ----------------
        states = {}
        with ExitStack() as sctx:
            sbp = sctx.enter_context(tc.tile_pool(name="setup", bufs=1))
            pst = sctx.enter_context(tc.tile_pool(name="pst", bufs=2, space="PSUM"))

            xt = sbp.tile([P, KBN, N], BF, tag="xt")
            yt = sbp.tile([P, KBN, N], BF, tag="yt")
            wqt = sbp.tile([P, KBN, NF], BF, tag="wqt")
            wkt = sbp.tile([P, KBN, NF], BF, tag="wkt")
            wvt = sbp.tile([P, KBN, NF], BF, tag="wvt")

            def proj_qk(m):
                for wt, out_sb in ((wqt, qt), (wkt, kt)):
                    for ch in range(2):
                        sl = slice(ch * 512, (ch + 1) * 512)
                        ps = pst.tile([P, 512], F32, tag="proj")
                        for kb in range(KBN):
                            nc.tensor.matmul(
                                ps[:],
                                lhsT=wt[:, kb, m * P:(m + 1) * P],
                                rhs=xt[:, kb, sl],
                                start=(kb == 0),
                                stop=(kb == KBN - 1),
                            )
                        nc.vector.tensor_copy(out_sb[:, m, sl], ps[:])

            # Loads cast f32->bf16 in-flight via SWDGE (gpsimd DMA): no f32
            # staging, no DVE cast pass. Loads and xbar transposes must not
            # interleave (the scheduler serializes every xbar mode switch),
            # so they are fenced into two groups. Early heads' phase 1 and
            # the projections fill the PE/ACT meanwhile.
            mats = ((x_d, NPT, xt), (wq_d, MB, wqt), (wk_d, MB, wkt),
                    (y_d, NPT, yt), (wv_d, MB, wvt))
            with ExitStack() as lctx:
                bfp = lctx.enter_context(tc.tile_pool(name="bfp", bufs=1))
                slabs = []
                for mi, (src_d, nslab, dst) in enumerate(mats):
                    bft = bfp.tile([P, nslab, D], BF, tag=f"bf{mi}",
                                   name=f"bf{mi}")
                    slabs.append(bft)

                def cast_loads(lo, hi):
                    for mi in range(lo, hi):
                        src_d, nslab, _ = mats[mi]
                        for s in range(nslab):
                            nc.gpsimd.dma_start(
                                out=slabs[mi][:, s, :],
                                in_=src_d[s * P:(s + 1) * P, :])

                def transposes(lo, hi):
                    for mi in range(lo, hi):
                        _, nslab, dst = mats[mi]
                        for s in range(nslab):
                            nc.scalar.dma_start_transpose(
                                out=dst[:, :, s * P:(s + 1) * P],
                                in_=slabs[mi][:, s, :])

                cast_loads(0, 3)          # x, wq, wk
                tc.no_sync_barrier()
                transposes(0, 3)
                tc.no_sync_barrier()
                cast_loads(3, 5)          # y, wv
                proj_qk(0)
                e1t0 = phase1a(0)
                e1t1 = phase1a(1)
                tc.no_sync_barrier()
                transposes(3, 5)
                proj_qk(1)
                states[0] = phase1b(0, e1t0)
                proj_qk(2)
                states[1] = phase1b(1, e1t1)
                proj_qk(3)
            for i in range(NPT):
                ps = pst.tile([P, 512], F32, tag="proj")
                for kb in range(KBN):
                    nc.tensor.matmul(
                        ps[:],
                        lhsT=yt[:, kb, i * P:(i + 1) * P],
                        rhs=wvt[:, kb, :],
                        start=(kb == 0),
                        stop=(kb == KBN - 1),
                    )
                nc.vector.tensor_copy(
                    vv[:, i, :, 0:DK],
                    ps[:].rearrange("p (h d) -> p h d", h=FH),
                )

        # ---------------- per-head attention ----------------
        e3p = ctx.enter_context(tc.tile_pool(name="e3p", bufs=4))
        e2p = ctx.enter_context(tc.tile_pool(name="e2p", bufs=4))
        osbp = ctx.enter_context(tc.tile_pool(name="osbp", bufs=4))
        otsp = ctx.enter_context(tc.tile_pool(name="otsp", bufs=4))
        smp = ctx.enter_context(tc.tile_pool(name="smp", bufs=24))
        pso = ctx.enter_context(tc.tile_pool(name="pso", bufs=2, space="PSUM"))

        def phase2(h, e1t, r1b):
            """dist mul, second exps, O matmuls, transpose-out, normalize."""
            r1x = r1b.rearrange("p (o n) -> p o n", o=1).broadcast_to([P, 2, N])
            e3s, e2s = [], []
            for half in range(4):
                d2 = e1t[:, half * 2:(half + 1) * 2, :]
                # D = dist (in place over E1T); later halves have slack, so
                # they go to gpsimd to keep the DVE free
                eng = nc.vector if half < 2 else nc.gpsimd
                eng.tensor_mul(d2, d2, r1x)
                e3 = e3p.tile([P, 2, N], BF, tag="e3", name=f"e3_{h}_{half}")
                nc.scalar.activation(e3[:], d2, EXP)
                e2 = e2p.tile([P, 2, N], BF, tag="e2", name=f"e2_{h}_{half}")
                if half < E2_ACT_BATCHES:
                    nc.scalar.activation(e2[:], d2, EXP, scale=-1.0)
                else:
                    nc.vector._custom_dve(
                        RECIPROCAL_APPROX_FAST, out=e2[:], in0=e3[:],
                        s0=cc["s0"], s1=cc["s1"], imm2=cc["imm2"])
                e3s.append(e3)
                e2s.append(e2)

            osb3 = osbp.tile([80, N], BF, tag="osb", name=f"osb3_{h}")
            osb2 = osbp.tile([80, N], BF, tag="osb", name=f"osb2_{h}")
            for ch in range(2):
                sl = slice(ch * 512, (ch + 1) * 512)
                o3_ps = pso.tile([DK + 1, 512], F32, tag="o", name=f"o3_{h}_{ch}")
                o2_ps = pso.tile([DK + 1, 512], F32, tag="o", name=f"o2_{h}_{ch}")
                for kb in range(KBN):
                    nc.tensor.matmul(
                        o3_ps[:], lhsT=vv[:, kb, h, :],
                        rhs=e3s[kb // 2][:, kb % 2, sl],
                        start=(kb == 0), stop=(kb == KBN - 1),
                    )
                    nc.tensor.matmul(
                        o2_ps[:], lhsT=vv[:, kb, h, :],
                        rhs=e2s[kb // 2][:, kb % 2, sl],
                        start=(kb == 0), stop=(kb == KBN - 1),
                    )
                nc.vector.tensor_copy(osb3[0:DK + 1, sl], o3_ps[:])
                nc.vector.tensor_copy(osb2[0:DK + 1, sl], o2_ps[:])

            for osb_t, out_t in ((osb3, att_sb), (osb2, catt_sb)):
                ot = otsp.tile([P, NPT, 80], BF, tag="ots", name=f"ot_{h}")
                nc.sync.dma_start_transpose(out=ot[:], in_=osb_t[:])
                # one batched reciprocal of the 8 rowsums, then per-tile
                # scalar multiplies; these are leaf ops, so half go to the
                # otherwise-idle gpsimd engine
                rr = smp.tile([P, NPT], F32, tag="rr", name=f"rr_{h}")
                nc.vector.reciprocal(rr[:], ot[:, :, DK])
                for i in range(NPT):
                    eng = nc.gpsimd if i % 2 == 0 else nc.vector
                    eng.tensor_scalar_mul(
                        out_t[:, i, h * DK:(h + 1) * DK], ot[:, i, 0:DK],
                        rr[:, i:i + 1])

        # software pipeline (heads 0-1 already ran phase 1 during setup):
        # phase2(h) is emitted just after phase1(h+2)'s work is in flight
        for h in range(2, FH):
            phase2(h - 2, *states.pop(h - 2))
            states[h] = phase1(h)
        phase2(FH - 2, *states.pop(FH - 2))
        phase2(FH - 1, *states.pop(FH - 1))

        for i in range(NPT):
            nc.sync.dma_start(out=att_d[i * P:(i + 1) * P, :], in_=att_sb[:, i, :])
            nc.sync.dma_start(out=catt_d[i * P:(i + 1) * P, :], in_=catt_sb[:, i, :])

    nc.finalize()
    return nc


_NC_CACHE = {}


def _get_nc():
    if "nc" not in _NC_CACHE:
        _NC_CACHE["nc"] = build_nc()
    return _NC_CACHE["nc"]


def _make_in_maps(x, y, Wq, Wk, Wv):
    x = np.ascontiguousarray(np.asarray(x, dtype=np.float32))
    y = np.ascontiguousarray(np.asarray(y, dtype=np.float32))
    Wq = np.ascontiguousarray(np.asarray(Wq, dtype=np.float32))
    Wk = np.ascontiguousarray(np.asarray(Wk, dtype=np.float32))
    Wv = np.ascontiguousarray(np.asarray(Wv, dtype=np.float32))
    in_maps = []
    for c in range(8):
        b, h0 = c // 2, (c % 2) * 8
        rows = slice(h0 * DK, h0 * DK + NF)
        in_maps.append({
            "x": x[b],
            "y": y[b],
            "wq": np.ascontiguousarray(Wq[rows]),
            "wk": np.ascontiguousarray(Wk[rows]),
            "wv": np.ascontiguousarray(Wv[rows]),
        })
    return in_maps


def run_cores(x, y, Wq, Wk, Wv, trace=False, tmpdir=None):
    nc = _get_nc()
    res = run_bass_kernel_spmd(
        nc, _make_in_maps(x, y, Wq, Wk, Wv), core_ids=list(range(8)),
        trace=trace, tmpdir=tmpdir,
    )
    B = 4
    c_att = np.empty((B, N, 2 * NF), dtype=np.float32)
    att = np.empty((B, N, 2 * NF), dtype=np.float32)
    for c, r in enumerate(res.results):
        b, cols = c // 2, slice((c % 2) * NF, (c % 2) * NF + NF)
        c_att[b][:, cols] = np.asarray(r["catt"], dtype=np.float32)
        att[b][:, cols] = np.asarray(r["att"], dtype=np.float32)
    return (c_att, att), res


def kernel(x, y, Wq, Wk, Wv):
    out, _ = run_cores(x, y, Wq, Wk, Wv)
    return out
